# revision 17
# baseline (speedup 1.0000x reference)
"""Izhikevich neuron simulation on 8 Trainium2 NeuronCores.

Problem: input_current [32, 2000, 512] f32 -> (spikes, voltages, recovery),
each [32, 2000, 512] f32, via a 2000-step sequential recurrence that is
independent per (batch, neuron) element.

Sharding: data-parallel over neurons - core k owns neurons [64k, 64(k+1)).
Per core the 32*64 = 2048 state elements live as a [128 partition, 16] tile.

The recurrence is executed as 3 custom DVE ops per step (all on the vector
engine, in program order, no cross-engine sync in the serial chain), using
re-encoded state so every op needs only 2 tensor streams and <= 3 scalars:

    zeta  = 0.2 * v_mid + 17.5          (v_mid = pre-reset voltage)
    omega = 100 * (u - 8*s_prev) + 1750 (recovery w/o last spike bump)

    X: x_b    = select(zeta >= 23.5, 12.25, zeta^2) - 0.01*omega
    W: omega' = 0.99*omega + select(zeta >= 23.5, 796.5, zeta)
    Z: zeta'  = (x_b + I_t)*0.1 + 2.625

Outputs are decoded in bulk per block of 125 steps, all three on the DVE
(gpsimd's is_ge path is ~16x slower, and any gpsimd streaming op running
concurrently with the chain stalls it via SBUF contention):
    s = (zeta' >= 23.5) ; v = select(s, -65, 5*zeta' - 87.5)
    u = 0.01*omega' + 8*s - 17.5
The next block's input DMA + K feed are emitted before this block's
decode so kblk is never queued behind decode work.
"""

import sys

if "/opt/trn_rl_repo" not in sys.path:
    sys.path.insert(0, "/opt/trn_rl_repo")

import numpy as np

# ---------------------------------------------------------------- problem dims
B, T, N = 32, 2000, 512
NCORES = 8
NSH = N // NCORES          # 64 neurons per core
P = 128                    # SBUF partitions
CPT = (B * NSH) // P       # 16 free elements per step-tile
TB = 125                   # steps per block
NBLK = T // TB             # 16 blocks
F32 = np.float32

_REG = {}                  # name -> DveOp, populated once
_NC_CACHE = {}             # built bass program cache


def _register_custom_ops():
    """Define and register the custom DVE ops (runtime-computed uop shas)."""
    if _REG:
        return _REG
    import concourse.dve_ops as dve_ops
    from concourse.dve_ops import DveOp
    from concourse.dve_spec import Spec, Src0, Src1, C0, C1, C2, sq, select, lower
    from concourse.dve_uop import DveOpSpec

    specs = {
        "IZH_W": Spec(
            body=Src1 * C0 + select(Src0 >= C1, C2, Src0),
            reference=lambda in0, in1, s0, s1, imm2: (
                in1 * s0 + np.where(in0 >= s1, imm2, in0)
            ).astype(np.float32),
        ),
        "IZH_ZX": Spec(
            body=(select(Src0 >= C0, C1, sq(Src0)) + Src1) * C2,
            reference=lambda in0, in1, s0, s1, imm2: (
                (np.where(in0 >= s0, s1, in0 * in0) + in1) * imm2
            ).astype(np.float32),
        ),
        "IZH_G": Spec(
            body=(Src0 + Src1) * C0,
            reference=lambda in0, in1, s0, s1, imm2: (
                (in0 + in1) * s0
            ).astype(np.float32),
        ),
        "IZH_VDEC": Spec(
            body=select(Src1, C2, Src0 * C0 - C1),
            reference=lambda in0, in1, s0, s1, imm2: np.where(
                in1 != 0, imm2, in0 * s0 - s1
            ).astype(np.float32),
        ),
        # s = (zeta >= 23.5)  — spike decode, 1-source
        "IZH_SDEC": Spec(
            body=Src0 >= C0,
            reference=lambda in0, in1, s0, s1, imm2: (in0 >= s0).astype(
                np.float32
            ),
        ),
        # u = 0.01*omega + 8*s - 17.5  (in0 = s, in1 = omega)
        "IZH_UDEC": Spec(
            body=Src1 * C0 + Src0 * C1 + C2,
            reference=lambda in0, in1, s0, s1, imm2: (
                in1 * s0 + in0 * s1 + imm2
            ).astype(np.float32),
        ),
    }

    for name, spec in specs.items():
        if name in dve_ops._SUB_OPCODE_FOR_NAME:
            _REG[name] = next(o for o in dve_ops.OPS if o.name == name)
            continue
        row = dve_ops._CUSTOM_DVE_ROW_BASE + len(dve_ops.OPS)
        assert row < 0x20, "custom DVE row budget exceeded"
        dve_ops._SUB_OPCODE_FOR_NAME[name] = row
        shas = {}
        for ver in ("v3", "v4"):
            s = DveOpSpec(
                name=name,
                opcode=row,
                uops=lower(spec, ver=ver),
                rd1_en=True,
            )
            shas[ver] = s.sha(ver)
        op = DveOp(name, spec, subdim=False, uops_sha=shas)
        dve_ops.OPS.append(op)
        dve_ops.CUSTOM_DVE_SPECS[name] = spec
        _REG[name] = op
    return _REG


def _build_bass(reps=1):
    """Build the per-core Bass/Tile program (identical for all 8 cores).

    reps > 1 repeats the whole computation back-to-back (timing only)."""
    if reps in _NC_CACHE:
        return _NC_CACHE[reps]

    import concourse.bacc as bacc
    import concourse.mybir as mybir
    import concourse.tile as tile
    from contextlib import ExitStack

    ops = _register_custom_ops()
    WOP, ZXOP, GOP = ops["IZH_W"], ops["IZH_ZX"], ops["IZH_G"]
    VDEC, SDEC, UDEC = ops["IZH_VDEC"], ops["IZH_SDEC"], ops["IZH_UDEC"]

    f32 = mybir.dt.float32
    nc = bacc.Bacc(
        "TRN2",
        target_bir_lowering=False,
        debug=False,
        enable_asserts=False,
        num_devices=NCORES,
    )

    inp = nc.dram_tensor("inp", [P, T * CPT], f32, kind="ExternalInput").ap()
    s_out = nc.dram_tensor("s_out", [P, T * CPT], f32, kind="ExternalOutput").ap()
    v_out = nc.dram_tensor("v_out", [P, T * CPT], f32, kind="ExternalOutput").ap()
    u_out = nc.dram_tensor("u_out", [P, T * CPT], f32, kind="ExternalOutput").ap()

    BW = TB * CPT  # block width in free elements
    ge = mybir.AluOpType.is_ge
    add = mybir.AluOpType.add
    mult = mybir.AluOpType.mult

    with tile.TileContext(nc) as tc, ExitStack() as ctx:
        iopool = ctx.enter_context(tc.tile_pool(name="io", bufs=3))
        spool = ctx.enter_context(tc.tile_pool(name="state", bufs=2))
        opool = ctx.enter_context(tc.tile_pool(name="outs", bufs=2))

        def emit_input(gi):
            """DMA input block gi and compute its K feed on gpsimd."""
            last = gi == NBLK - 1
            # input block: columns [gi*TB, gi*TB + TB] inclusive when
            # possible (one extra column feeds the shifted K-block)
            iw = BW if last else BW + CPT
            iblk = iopool.tile([P, BW + CPT], f32, tag="iblk")
            nc.sync.dma_start(
                out=iblk[:, 0:iw], in_=inp[:, gi * BW:gi * BW + iw]
            )
            # K_t = -100*I_{t+1} - 2625  (gamma feed; on GPSIMD)
            kblk = iopool.tile([P, BW], f32, tag="kblk")
            kw = BW - CPT if last else BW
            nc.gpsimd.tensor_scalar(
                kblk[:, 0:kw], iblk[:, CPT:CPT + kw], -100.0, -2625.0, mult, add
            )
            if last:  # value never consumed; keep it finite
                nc.gpsimd.tensor_scalar(
                    kblk[:, kw:BW], iblk[:, kw:BW], -100.0, -2625.0, mult, add
                )
            return iblk, kblk

        # --- block-0 preamble, ordered so the chain starts ASAP:
        # memsets (no deps) -> gblk init (waits head DMA only) -> kblk head
        # -> kblk rest. The chain's first ops then wait ~3.6us, not ~6.8us.
        HEAD = 16 * CPT  # 16 steps of chain headroom
        iblk0 = iopool.tile([P, BW + CPT], f32, tag="iblk")
        nc.sync.dma_start(out=iblk0[:, 0:HEAD], in_=inp[:, 0:HEAD])
        nc.sync.dma_start(out=iblk0[:, HEAD:BW + CPT], in_=inp[:, HEAD:BW + CPT])
        kblk0 = iopool.tile([P, BW], f32, tag="kblk")
        zblk0 = spool.tile([P, BW + CPT], f32, tag="zblk")
        wblk0 = spool.tile([P, BW + CPT], f32, tag="wblk")
        gblk0 = spool.tile([P, BW + CPT], f32, tag="gblk")
        nc.gpsimd.memset(zblk0[:, 0:CPT], 4.5)
        nc.gpsimd.memset(wblk0[:, 0:CPT], 450.0)
        # gamma_0 = I_0 + 21.75
        nc.gpsimd.tensor_scalar(gblk0[:, 0:CPT], iblk0[:, 0:CPT], 21.75, None, add)
        nc.gpsimd.tensor_scalar(
            kblk0[:, 0:HEAD - CPT], iblk0[:, CPT:HEAD], -100.0, -2625.0, mult, add
        )
        nc.gpsimd.tensor_scalar(
            kblk0[:, HEAD - CPT:BW], iblk0[:, HEAD:BW + CPT], -100.0, -2625.0,
            mult, add,
        )

        prev_z = prev_w = prev_g = None
        nxt = (iblk0, kblk0)
        NTOT = NBLK * reps
        for g in range(NTOT):
            gi = g % NBLK
            iblk, kblk = nxt

            if g == 0:
                zblk, wblk, gblk = zblk0, wblk0, gblk0
            else:
                zblk = spool.tile([P, BW + CPT], f32, tag="zblk")
                wblk = spool.tile([P, BW + CPT], f32, tag="wblk")
                gblk = spool.tile([P, BW + CPT], f32, tag="gblk")
                nc.scalar.copy(zblk[:, 0:CPT], prev_z[:, BW:BW + CPT])
                nc.scalar.copy(wblk[:, 0:CPT], prev_w[:, BW:BW + CPT])
                nc.scalar.copy(gblk[:, 0:CPT], prev_g[:, BW:BW + CPT])


            for j in range(TB):
                c0, c1, c2 = j * CPT, (j + 1) * CPT, (j + 2) * CPT
                zp, wp, gp = zblk[:, c0:c1], wblk[:, c0:c1], gblk[:, c0:c1]
                zn, wn, gn = zblk[:, c1:c2], wblk[:, c1:c2], gblk[:, c1:c2]
                # order (W, ZX, G): every consumer is >=2 ops after its
                # producer, hiding the DVE write-ack latency
                nc.vector._custom_dve(
                    WOP, out=wn, in0=zp, in1=wp, s0=0.99, s1=23.5, imm2=796.5
                )
                nc.vector._custom_dve(
                    ZXOP, out=zn, in0=zp, in1=gp, s0=23.5, s1=12.25, imm2=0.1
                )
                nc.vector._custom_dve(GOP, out=gn, in0=wn, in1=kblk[:, c0:c1], s0=-0.01)

            # prefetch next block's input + K feed BEFORE this block's
            # decode ops so kblk is ahead of them in the gpsimd queue
            if g + 1 < NTOT:
                nxt = emit_input((g + 1) % NBLK)

            zdat = zblk[:, CPT:BW + CPT]
            wdat = wblk[:, CPT:BW + CPT]
            # spike decode on DVE (gpsimd is_ge runs ~16x slower)
            # all three decodes serially on the DVE: concurrent gpsimd
            # streaming ops stall the chain via SBUF contention (measured),
            # so serial-on-vector is net faster. Last block is chunked so
            # the final DMA drain overlaps the remaining decode.
            sblk = opool.tile([P, BW], f32, tag="sblk")
            vblk = opool.tile([P, BW], f32, tag="vblk")
            ublk = opool.tile([P, BW], f32, tag="ublk")
            nch = 4 if g == NTOT - 1 else 1
            cw = BW // nch
            for h in range(nch):
                cs = slice(h * cw, (h + 1) * cw)
                zd = zblk[:, CPT + h * cw:CPT + (h + 1) * cw]
                wd = wblk[:, CPT + h * cw:CPT + (h + 1) * cw]
                nc.vector._custom_dve(SDEC, out=sblk[:, cs], in0=zd, s0=23.5)
                nc.vector._custom_dve(
                    VDEC, out=vblk[:, cs], in0=zd, in1=sblk[:, cs],
                    s0=5.0, s1=87.5, imm2=-65.0,
                )
                nc.vector._custom_dve(
                    UDEC, out=ublk[:, cs], in0=sblk[:, cs], in1=wd,
                    s0=0.01, s1=8.0, imm2=-17.5,
                )
                ob = slice(gi * BW + h * cw, gi * BW + (h + 1) * cw)
                nc.sync.dma_start(out=s_out[:, ob], in_=sblk[:, cs])
                nc.sync.dma_start(out=v_out[:, ob], in_=vblk[:, cs])
                nc.sync.dma_start(out=u_out[:, ob], in_=ublk[:, cs])

            prev_z, prev_w, prev_g = zblk, wblk, gblk

    nc.compile()
    _NC_CACHE[reps] = nc
    return nc


def _stage(shard):
    """[32, 2000, 64] -> [128, 2000*16]; (b, j) -> partition b*4 + j//16."""
    return np.ascontiguousarray(
        shard.reshape(B, T, 4, 16).transpose(0, 2, 1, 3).reshape(P, T * CPT)
    )


def _unstage(arr):
    """[128, 2000*16] -> [32, 2000, 64] (inverse of _stage)."""
    return arr.reshape(B, 4, T, 16).transpose(0, 2, 1, 3).reshape(B, T, NSH)


def kernel(input_current):
    from concourse.bass_utils import run_bass_kernel_spmd

    input_current = np.asarray(input_current, dtype=F32)
    assert input_current.shape == (B, T, N)

    nc = _build_bass()
    in_maps = [
        {"inp": _stage(input_current[:, :, k * NSH:(k + 1) * NSH])}
        for k in range(NCORES)
    ]
    res = run_bass_kernel_spmd(nc, in_maps, list(range(NCORES)))

    spikes = np.empty((B, T, N), F32)
    volts = np.empty((B, T, N), F32)
    recov = np.empty((B, T, N), F32)
    for k in range(NCORES):
        sl = slice(k * NSH, (k + 1) * NSH)
        spikes[:, :, sl] = _unstage(res.results[k]["s_out"])
        volts[:, :, sl] = _unstage(res.results[k]["v_out"])
        recov[:, :, sl] = _unstage(res.results[k]["u_out"])
    return spikes, volts, recov



# revision 18
# speedup vs baseline: 1.0027x; 1.0027x over previous
"""Izhikevich neuron simulation on 8 Trainium2 NeuronCores.

Problem: input_current [32, 2000, 512] f32 -> (spikes, voltages, recovery),
each [32, 2000, 512] f32, via a 2000-step sequential recurrence that is
independent per (batch, neuron) element.

Sharding: data-parallel over neurons - core k owns neurons [64k, 64(k+1)).
Per core the 32*64 = 2048 state elements live as a [128 partition, 16] tile.

The recurrence is executed as 3 custom DVE ops per step (all on the vector
engine, in program order, no cross-engine sync in the serial chain), using
re-encoded state so every op needs only 2 tensor streams and <= 3 scalars:

    zeta  = 0.2 * v_mid + 17.5          (v_mid = pre-reset voltage)
    omega = 100 * (u - 8*s_prev) + 1750 (recovery w/o last spike bump)

    X: x_b    = select(zeta >= 23.5, 12.25, zeta^2) - 0.01*omega
    W: omega' = 0.99*omega + select(zeta >= 23.5, 796.5, zeta)
    Z: zeta'  = (x_b + I_t)*0.1 + 2.625

Outputs are decoded in bulk per block of 125 steps, all three on the DVE
(gpsimd's is_ge path is ~16x slower, and any gpsimd streaming op running
concurrently with the chain stalls it via SBUF contention):
    s = (zeta' >= 23.5) ; v = select(s, -65, 5*zeta' - 87.5)
    u = 0.01*omega' + 8*s - 17.5
The next block's input DMA + K feed are emitted before this block's
decode so kblk is never queued behind decode work.
"""

import sys

if "/opt/trn_rl_repo" not in sys.path:
    sys.path.insert(0, "/opt/trn_rl_repo")

import numpy as np

# ---------------------------------------------------------------- problem dims
B, T, N = 32, 2000, 512
NCORES = 8
NSH = N // NCORES          # 64 neurons per core
P = 128                    # SBUF partitions
CPT = (B * NSH) // P       # 16 free elements per step-tile
TB = 125                   # steps per block
NBLK = T // TB             # 16 blocks
F32 = np.float32

_REG = {}                  # name -> DveOp, populated once
_NC_CACHE = {}             # built bass program cache


def _register_custom_ops():
    """Define and register the custom DVE ops (runtime-computed uop shas)."""
    if _REG:
        return _REG
    import concourse.dve_ops as dve_ops
    from concourse.dve_ops import DveOp
    from concourse.dve_spec import Spec, Src0, Src1, C0, C1, C2, sq, select, lower
    from concourse.dve_uop import DveOpSpec

    specs = {
        "IZH_W": Spec(
            body=Src1 * C0 + select(Src0 >= C1, C2, Src0),
            reference=lambda in0, in1, s0, s1, imm2: (
                in1 * s0 + np.where(in0 >= s1, imm2, in0)
            ).astype(np.float32),
        ),
        "IZH_ZX": Spec(
            body=(select(Src0 >= C0, C1, sq(Src0)) + Src1) * C2,
            reference=lambda in0, in1, s0, s1, imm2: (
                (np.where(in0 >= s0, s1, in0 * in0) + in1) * imm2
            ).astype(np.float32),
        ),
        "IZH_G": Spec(
            body=(Src0 + Src1) * C0,
            reference=lambda in0, in1, s0, s1, imm2: (
                (in0 + in1) * s0
            ).astype(np.float32),
        ),
        "IZH_VDEC": Spec(
            body=select(Src1, C2, Src0 * C0 - C1),
            reference=lambda in0, in1, s0, s1, imm2: np.where(
                in1 != 0, imm2, in0 * s0 - s1
            ).astype(np.float32),
        ),
        # s = (zeta >= 23.5)  — spike decode, 1-source
        "IZH_SDEC": Spec(
            body=Src0 >= C0,
            reference=lambda in0, in1, s0, s1, imm2: (in0 >= s0).astype(
                np.float32
            ),
        ),
        # u = 0.01*omega + 8*s - 17.5  (in0 = s, in1 = omega)
        "IZH_UDEC": Spec(
            body=Src1 * C0 + Src0 * C1 + C2,
            reference=lambda in0, in1, s0, s1, imm2: (
                in1 * s0 + in0 * s1 + imm2
            ).astype(np.float32),
        ),
    }

    for name, spec in specs.items():
        if name in dve_ops._SUB_OPCODE_FOR_NAME:
            _REG[name] = next(o for o in dve_ops.OPS if o.name == name)
            continue
        row = dve_ops._CUSTOM_DVE_ROW_BASE + len(dve_ops.OPS)
        assert row < 0x20, "custom DVE row budget exceeded"
        dve_ops._SUB_OPCODE_FOR_NAME[name] = row
        shas = {}
        for ver in ("v3", "v4"):
            s = DveOpSpec(
                name=name,
                opcode=row,
                uops=lower(spec, ver=ver),
                rd1_en=True,
            )
            shas[ver] = s.sha(ver)
        op = DveOp(name, spec, subdim=False, uops_sha=shas)
        dve_ops.OPS.append(op)
        dve_ops.CUSTOM_DVE_SPECS[name] = spec
        _REG[name] = op
    return _REG


def _build_bass(reps=1):
    """Build the per-core Bass/Tile program (identical for all 8 cores).

    reps > 1 repeats the whole computation back-to-back (timing only)."""
    if reps in _NC_CACHE:
        return _NC_CACHE[reps]

    import concourse.bacc as bacc
    import concourse.mybir as mybir
    import concourse.tile as tile
    from contextlib import ExitStack

    ops = _register_custom_ops()
    WOP, ZXOP, GOP = ops["IZH_W"], ops["IZH_ZX"], ops["IZH_G"]
    VDEC, SDEC, UDEC = ops["IZH_VDEC"], ops["IZH_SDEC"], ops["IZH_UDEC"]

    f32 = mybir.dt.float32
    nc = bacc.Bacc(
        "TRN2",
        target_bir_lowering=False,
        debug=False,
        enable_asserts=False,
        num_devices=NCORES,
    )

    inp = nc.dram_tensor("inp", [P, T * CPT], f32, kind="ExternalInput").ap()
    s_out = nc.dram_tensor("s_out", [P, T * CPT], f32, kind="ExternalOutput").ap()
    v_out = nc.dram_tensor("v_out", [P, T * CPT], f32, kind="ExternalOutput").ap()
    u_out = nc.dram_tensor("u_out", [P, T * CPT], f32, kind="ExternalOutput").ap()

    BW = TB * CPT  # block width in free elements
    ge = mybir.AluOpType.is_ge
    add = mybir.AluOpType.add
    mult = mybir.AluOpType.mult

    with tile.TileContext(nc) as tc, ExitStack() as ctx:
        iopool = ctx.enter_context(tc.tile_pool(name="io", bufs=3))
        spool = ctx.enter_context(tc.tile_pool(name="state", bufs=2))
        opool = ctx.enter_context(tc.tile_pool(name="outs", bufs=2))

        def emit_input(gi):
            """DMA input block gi and compute its K feed on gpsimd."""
            last = gi == NBLK - 1
            # input block: columns [gi*TB, gi*TB + TB] inclusive when
            # possible (one extra column feeds the shifted K-block)
            iw = BW if last else BW + CPT
            iblk = iopool.tile([P, BW + CPT], f32, tag="iblk")
            nc.sync.dma_start(
                out=iblk[:, 0:iw], in_=inp[:, gi * BW:gi * BW + iw]
            )
            # K_t = -100*I_{t+1} - 2625  (gamma feed; on GPSIMD)
            kblk = iopool.tile([P, BW], f32, tag="kblk")
            kw = BW - CPT if last else BW
            nc.gpsimd.tensor_scalar(
                kblk[:, 0:kw], iblk[:, CPT:CPT + kw], -100.0, -2625.0, mult, add
            )
            if last:  # value never consumed; keep it finite
                nc.gpsimd.tensor_scalar(
                    kblk[:, kw:BW], iblk[:, kw:BW], -100.0, -2625.0, mult, add
                )
            return iblk, kblk

        def emit_input_first():
            """Block 0 with the input DMA split so the chain starts after
            only the head lands instead of the full 1MB block."""
            HEAD = 16 * CPT  # 16 steps of chain headroom
            iblk = iopool.tile([P, BW + CPT], f32, tag="iblk")
            nc.sync.dma_start(out=iblk[:, 0:HEAD], in_=inp[:, 0:HEAD])
            nc.sync.dma_start(
                out=iblk[:, HEAD:BW + CPT], in_=inp[:, HEAD:BW + CPT]
            )
            kblk = iopool.tile([P, BW], f32, tag="kblk")
            nc.gpsimd.tensor_scalar(
                kblk[:, 0:HEAD - CPT], iblk[:, CPT:HEAD], -100.0, -2625.0,
                mult, add,
            )
            nc.gpsimd.tensor_scalar(
                kblk[:, HEAD - CPT:BW], iblk[:, HEAD:BW + CPT], -100.0,
                -2625.0, mult, add,
            )
            return iblk, kblk

        prev_z = prev_w = prev_g = None
        nxt = emit_input_first()  # prefetch block 0, head-split
        NTOT = NBLK * reps
        for g in range(NTOT):
            gi = g % NBLK
            iblk, kblk = nxt

            zblk = spool.tile([P, BW + CPT], f32, tag="zblk")
            wblk = spool.tile([P, BW + CPT], f32, tag="wblk")
            gblk = spool.tile([P, BW + CPT], f32, tag="gblk")
            if g == 0:
                nc.gpsimd.memset(zblk[:, 0:CPT], 4.5)
                nc.gpsimd.memset(wblk[:, 0:CPT], 450.0)
                # gamma_0 = I_0 + 21.75
                nc.gpsimd.tensor_scalar(
                    gblk[:, 0:CPT], iblk[:, 0:CPT], 21.75, None, add
                )
            else:
                nc.scalar.copy(zblk[:, 0:CPT], prev_z[:, BW:BW + CPT])
                nc.scalar.copy(wblk[:, 0:CPT], prev_w[:, BW:BW + CPT])
                nc.scalar.copy(gblk[:, 0:CPT], prev_g[:, BW:BW + CPT])


            for j in range(TB):
                c0, c1, c2 = j * CPT, (j + 1) * CPT, (j + 2) * CPT
                zp, wp, gp = zblk[:, c0:c1], wblk[:, c0:c1], gblk[:, c0:c1]
                zn, wn, gn = zblk[:, c1:c2], wblk[:, c1:c2], gblk[:, c1:c2]
                # order (W, ZX, G): every consumer is >=2 ops after its
                # producer, hiding the DVE write-ack latency
                nc.vector._custom_dve(
                    WOP, out=wn, in0=zp, in1=wp, s0=0.99, s1=23.5, imm2=796.5
                )
                nc.vector._custom_dve(
                    ZXOP, out=zn, in0=zp, in1=gp, s0=23.5, s1=12.25, imm2=0.1
                )
                nc.vector._custom_dve(GOP, out=gn, in0=wn, in1=kblk[:, c0:c1], s0=-0.01)

            # prefetch next block's input + K feed BEFORE this block's
            # decode ops so kblk is ahead of them in the gpsimd queue
            if g + 1 < NTOT:
                nxt = emit_input((g + 1) % NBLK)

            zdat = zblk[:, CPT:BW + CPT]
            wdat = wblk[:, CPT:BW + CPT]
            # spike decode on DVE (gpsimd is_ge runs ~16x slower)
            # all three decodes serially on the DVE: concurrent gpsimd
            # streaming ops stall the chain via SBUF contention (measured),
            # so serial-on-vector is net faster. Last block is chunked so
            # the final DMA drain overlaps the remaining decode.
            sblk = opool.tile([P, BW], f32, tag="sblk")
            vblk = opool.tile([P, BW], f32, tag="vblk")
            ublk = opool.tile([P, BW], f32, tag="ublk")
            nch = 4 if g == NTOT - 1 else 1
            cw = BW // nch
            for h in range(nch):
                cs = slice(h * cw, (h + 1) * cw)
                zd = zblk[:, CPT + h * cw:CPT + (h + 1) * cw]
                wd = wblk[:, CPT + h * cw:CPT + (h + 1) * cw]
                nc.vector._custom_dve(SDEC, out=sblk[:, cs], in0=zd, s0=23.5)
                nc.vector._custom_dve(
                    VDEC, out=vblk[:, cs], in0=zd, in1=sblk[:, cs],
                    s0=5.0, s1=87.5, imm2=-65.0,
                )
                nc.vector._custom_dve(
                    UDEC, out=ublk[:, cs], in0=sblk[:, cs], in1=wd,
                    s0=0.01, s1=8.0, imm2=-17.5,
                )
                ob = slice(gi * BW + h * cw, gi * BW + (h + 1) * cw)
                nc.sync.dma_start(out=s_out[:, ob], in_=sblk[:, cs])
                nc.sync.dma_start(out=v_out[:, ob], in_=vblk[:, cs])
                nc.sync.dma_start(out=u_out[:, ob], in_=ublk[:, cs])

            prev_z, prev_w, prev_g = zblk, wblk, gblk

    nc.compile()
    _NC_CACHE[reps] = nc
    return nc


def _stage(shard):
    """[32, 2000, 64] -> [128, 2000*16]; (b, j) -> partition b*4 + j//16."""
    return np.ascontiguousarray(
        shard.reshape(B, T, 4, 16).transpose(0, 2, 1, 3).reshape(P, T * CPT)
    )


def _unstage(arr):
    """[128, 2000*16] -> [32, 2000, 64] (inverse of _stage)."""
    return arr.reshape(B, 4, T, 16).transpose(0, 2, 1, 3).reshape(B, T, NSH)


def kernel(input_current):
    from concourse.bass_utils import run_bass_kernel_spmd

    input_current = np.asarray(input_current, dtype=F32)
    assert input_current.shape == (B, T, N)

    nc = _build_bass()
    in_maps = [
        {"inp": _stage(input_current[:, :, k * NSH:(k + 1) * NSH])}
        for k in range(NCORES)
    ]
    res = run_bass_kernel_spmd(nc, in_maps, list(range(NCORES)))

    spikes = np.empty((B, T, N), F32)
    volts = np.empty((B, T, N), F32)
    recov = np.empty((B, T, N), F32)
    for k in range(NCORES):
        sl = slice(k * NSH, (k + 1) * NSH)
        spikes[:, :, sl] = _unstage(res.results[k]["s_out"])
        volts[:, :, sl] = _unstage(res.results[k]["v_out"])
        recov[:, :, sl] = _unstage(res.results[k]["u_out"])
    return spikes, volts, recov



# revision 20
# speedup vs baseline: 1.0036x; 1.0009x over previous
"""Izhikevich neuron simulation on 8 Trainium2 NeuronCores.

Problem: input_current [32, 2000, 512] f32 -> (spikes, voltages, recovery),
each [32, 2000, 512] f32, via a 2000-step sequential recurrence that is
independent per (batch, neuron) element.

Sharding: data-parallel over neurons - core k owns neurons [64k, 64(k+1)).
Per core the 32*64 = 2048 state elements live as a [128 partition, 16] tile.

The recurrence is executed as 3 custom DVE ops per step (all on the vector
engine, in program order, no cross-engine sync in the serial chain), using
re-encoded state so every op needs only 2 tensor streams and <= 3 scalars:

    zeta  = 0.2 * v_mid + 17.5          (v_mid = pre-reset voltage)
    omega = 100 * (u - 8*s_prev) + 1750 (recovery w/o last spike bump)

    X: x_b    = select(zeta >= 23.5, 12.25, zeta^2) - 0.01*omega
    W: omega' = 0.99*omega + select(zeta >= 23.5, 796.5, zeta)
    Z: zeta'  = (x_b + I_t)*0.1 + 2.625

Outputs are decoded in bulk per block of 125 steps, all three on the DVE
(gpsimd's is_ge path is ~16x slower, and any gpsimd streaming op running
concurrently with the chain stalls it via SBUF contention):
    s = (zeta' >= 23.5) ; v = select(s, -65, 5*zeta' - 87.5)
    u = 0.01*omega' + 8*s - 17.5
The next block's input DMA + K feed are emitted before this block's
decode so kblk is never queued behind decode work.
"""

import sys

if "/opt/trn_rl_repo" not in sys.path:
    sys.path.insert(0, "/opt/trn_rl_repo")

import numpy as np

# ---------------------------------------------------------------- problem dims
B, T, N = 32, 2000, 512
NCORES = 8
NSH = N // NCORES          # 64 neurons per core
P = 128                    # SBUF partitions
CPT = (B * NSH) // P       # 16 free elements per step-tile
TB = 125                   # steps per block
NBLK = T // TB             # 16 blocks
F32 = np.float32

_REG = {}                  # name -> DveOp, populated once
_NC_CACHE = {}             # built bass program cache


def _register_custom_ops():
    """Define and register the custom DVE ops (runtime-computed uop shas)."""
    if _REG:
        return _REG
    import concourse.dve_ops as dve_ops
    from concourse.dve_ops import DveOp
    from concourse.dve_spec import Spec, Src0, Src1, C0, C1, C2, sq, select, lower
    from concourse.dve_uop import DveOpSpec

    specs = {
        "IZH_W": Spec(
            body=Src1 * C0 + select(Src0 >= C1, C2, Src0),
            reference=lambda in0, in1, s0, s1, imm2: (
                in1 * s0 + np.where(in0 >= s1, imm2, in0)
            ).astype(np.float32),
        ),
        "IZH_ZX": Spec(
            body=(select(Src0 >= C0, C1, sq(Src0)) + Src1) * C2,
            reference=lambda in0, in1, s0, s1, imm2: (
                (np.where(in0 >= s0, s1, in0 * in0) + in1) * imm2
            ).astype(np.float32),
        ),
        "IZH_G": Spec(
            body=(Src0 + Src1) * C0,
            reference=lambda in0, in1, s0, s1, imm2: (
                (in0 + in1) * s0
            ).astype(np.float32),
        ),
        "IZH_VDEC": Spec(
            body=select(Src1, C2, Src0 * C0 - C1),
            reference=lambda in0, in1, s0, s1, imm2: np.where(
                in1 != 0, imm2, in0 * s0 - s1
            ).astype(np.float32),
        ),
        # s = (zeta >= 23.5)  — spike decode, 1-source
        "IZH_SDEC": Spec(
            body=Src0 >= C0,
            reference=lambda in0, in1, s0, s1, imm2: (in0 >= s0).astype(
                np.float32
            ),
        ),
        # u = 0.01*omega + 8*s - 17.5  (in0 = s, in1 = omega)
        "IZH_UDEC": Spec(
            body=Src1 * C0 + Src0 * C1 + C2,
            reference=lambda in0, in1, s0, s1, imm2: (
                in1 * s0 + in0 * s1 + imm2
            ).astype(np.float32),
        ),
    }

    for name, spec in specs.items():
        if name in dve_ops._SUB_OPCODE_FOR_NAME:
            _REG[name] = next(o for o in dve_ops.OPS if o.name == name)
            continue
        row = dve_ops._CUSTOM_DVE_ROW_BASE + len(dve_ops.OPS)
        assert row < 0x20, "custom DVE row budget exceeded"
        dve_ops._SUB_OPCODE_FOR_NAME[name] = row
        shas = {}
        for ver in ("v3", "v4"):
            s = DveOpSpec(
                name=name,
                opcode=row,
                uops=lower(spec, ver=ver),
                rd1_en=True,
            )
            shas[ver] = s.sha(ver)
        op = DveOp(name, spec, subdim=False, uops_sha=shas)
        dve_ops.OPS.append(op)
        dve_ops.CUSTOM_DVE_SPECS[name] = spec
        _REG[name] = op
    return _REG


def _build_bass(reps=1):
    """Build the per-core Bass/Tile program (identical for all 8 cores).

    reps > 1 repeats the whole computation back-to-back (timing only)."""
    if reps in _NC_CACHE:
        return _NC_CACHE[reps]

    import concourse.bacc as bacc
    import concourse.mybir as mybir
    import concourse.tile as tile
    from contextlib import ExitStack

    ops = _register_custom_ops()
    WOP, ZXOP, GOP = ops["IZH_W"], ops["IZH_ZX"], ops["IZH_G"]
    VDEC, SDEC, UDEC = ops["IZH_VDEC"], ops["IZH_SDEC"], ops["IZH_UDEC"]

    f32 = mybir.dt.float32
    nc = bacc.Bacc(
        "TRN2",
        target_bir_lowering=False,
        debug=False,
        enable_asserts=False,
        num_devices=NCORES,
    )

    inp = nc.dram_tensor("inp", [P, T * CPT], f32, kind="ExternalInput").ap()
    s_out = nc.dram_tensor("s_out", [P, T * CPT], f32, kind="ExternalOutput").ap()
    v_out = nc.dram_tensor("v_out", [P, T * CPT], f32, kind="ExternalOutput").ap()
    u_out = nc.dram_tensor("u_out", [P, T * CPT], f32, kind="ExternalOutput").ap()

    BW = TB * CPT  # block width in free elements
    ge = mybir.AluOpType.is_ge
    add = mybir.AluOpType.add
    mult = mybir.AluOpType.mult

    with tile.TileContext(nc) as tc, ExitStack() as ctx:
        iopool = ctx.enter_context(tc.tile_pool(name="io", bufs=3))
        spool = ctx.enter_context(tc.tile_pool(name="state", bufs=2))
        opool = ctx.enter_context(tc.tile_pool(name="outs", bufs=2))

        def emit_input(gi):
            """DMA input block gi and compute its K feed on gpsimd."""
            last = gi == NBLK - 1
            # input block: columns [gi*TB, gi*TB + TB] inclusive when
            # possible (one extra column feeds the shifted K-block)
            iw = BW if last else BW + CPT
            iblk = iopool.tile([P, BW + CPT], f32, tag="iblk")
            nc.sync.dma_start(
                out=iblk[:, 0:iw], in_=inp[:, gi * BW:gi * BW + iw]
            )
            # K_t = -100*I_{t+1} - 2625  (gamma feed; on GPSIMD)
            kblk = iopool.tile([P, BW], f32, tag="kblk")
            kw = BW - CPT if last else BW
            nc.gpsimd.tensor_scalar(
                kblk[:, 0:kw], iblk[:, CPT:CPT + kw], -100.0, -2625.0, mult, add
            )
            if last:  # value never consumed; keep it finite
                nc.gpsimd.tensor_scalar(
                    kblk[:, kw:BW], iblk[:, kw:BW], -100.0, -2625.0, mult, add
                )
            return iblk, kblk

        def emit_input_first():
            """Block 0 with the input DMA split so the chain starts after
            only the head lands instead of the full 1MB block."""
            HEAD = 16 * CPT  # 16 steps of chain headroom
            iblk = iopool.tile([P, BW + CPT], f32, tag="iblk")
            nc.sync.dma_start(out=iblk[:, 0:HEAD], in_=inp[:, 0:HEAD])
            nc.sync.dma_start(
                out=iblk[:, HEAD:BW + CPT], in_=inp[:, HEAD:BW + CPT]
            )
            kblk = iopool.tile([P, BW], f32, tag="kblk")
            nc.gpsimd.tensor_scalar(
                kblk[:, 0:HEAD - CPT], iblk[:, CPT:HEAD], -100.0, -2625.0,
                mult, add,
            )
            nc.gpsimd.tensor_scalar(
                kblk[:, HEAD - CPT:BW], iblk[:, HEAD:BW + CPT], -100.0,
                -2625.0, mult, add,
            )
            return iblk, kblk

        prev_z = prev_w = prev_g = None
        nxt = emit_input_first()  # prefetch block 0, head-split
        NTOT = NBLK * reps
        for g in range(NTOT):
            gi = g % NBLK
            iblk, kblk = nxt

            zblk = spool.tile([P, BW + CPT], f32, tag="zblk")
            wblk = spool.tile([P, BW + CPT], f32, tag="wblk")
            gblk = spool.tile([P, BW + CPT], f32, tag="gblk")
            if g == 0:
                nc.gpsimd.memset(zblk[:, 0:CPT], 4.5)
                nc.gpsimd.memset(wblk[:, 0:CPT], 450.0)
                # gamma_0 = I_0 + 21.75
                nc.gpsimd.tensor_scalar(
                    gblk[:, 0:CPT], iblk[:, 0:CPT], 21.75, None, add
                )
            # g > 0: no carry-in copies — step 0 reads the previous block's
            # last column directly (the new tiles' column 0 stays unused),
            # removing 3 scalar copies + their standalone sem waits per block


            for j in range(TB):
                c0, c1, c2 = j * CPT, (j + 1) * CPT, (j + 2) * CPT
                if j == 0 and g > 0:
                    zp = prev_z[:, BW:BW + CPT]
                    wp = prev_w[:, BW:BW + CPT]
                    gp = prev_g[:, BW:BW + CPT]
                else:
                    zp, wp, gp = zblk[:, c0:c1], wblk[:, c0:c1], gblk[:, c0:c1]
                zn, wn, gn = zblk[:, c1:c2], wblk[:, c1:c2], gblk[:, c1:c2]
                # order (W, ZX, G): every consumer is >=2 ops after its
                # producer, hiding the DVE write-ack latency
                nc.vector._custom_dve(
                    WOP, out=wn, in0=zp, in1=wp, s0=0.99, s1=23.5, imm2=796.5
                )
                nc.vector._custom_dve(
                    ZXOP, out=zn, in0=zp, in1=gp, s0=23.5, s1=12.25, imm2=0.1
                )
                nc.vector._custom_dve(GOP, out=gn, in0=wn, in1=kblk[:, c0:c1], s0=-0.01)

            # prefetch next block's input + K feed BEFORE this block's
            # decode ops so kblk is ahead of them in the gpsimd queue
            if g + 1 < NTOT:
                nxt = emit_input((g + 1) % NBLK)

            zdat = zblk[:, CPT:BW + CPT]
            wdat = wblk[:, CPT:BW + CPT]
            # spike decode on DVE (gpsimd is_ge runs ~16x slower)
            # all three decodes serially on the DVE: concurrent gpsimd
            # streaming ops stall the chain via SBUF contention (measured),
            # so serial-on-vector is net faster. Last block is chunked so
            # the final DMA drain overlaps the remaining decode.
            sblk = opool.tile([P, BW], f32, tag="sblk")
            vblk = opool.tile([P, BW], f32, tag="vblk")
            ublk = opool.tile([P, BW], f32, tag="ublk")
            nch = 4 if g == NTOT - 1 else 1
            cw = BW // nch
            for h in range(nch):
                cs = slice(h * cw, (h + 1) * cw)
                zd = zblk[:, CPT + h * cw:CPT + (h + 1) * cw]
                wd = wblk[:, CPT + h * cw:CPT + (h + 1) * cw]
                nc.vector._custom_dve(SDEC, out=sblk[:, cs], in0=zd, s0=23.5)
                nc.vector._custom_dve(
                    VDEC, out=vblk[:, cs], in0=zd, in1=sblk[:, cs],
                    s0=5.0, s1=87.5, imm2=-65.0,
                )
                nc.vector._custom_dve(
                    UDEC, out=ublk[:, cs], in0=sblk[:, cs], in1=wd,
                    s0=0.01, s1=8.0, imm2=-17.5,
                )
                ob = slice(gi * BW + h * cw, gi * BW + (h + 1) * cw)
                nc.sync.dma_start(out=s_out[:, ob], in_=sblk[:, cs])
                nc.sync.dma_start(out=v_out[:, ob], in_=vblk[:, cs])
                nc.sync.dma_start(out=u_out[:, ob], in_=ublk[:, cs])

            prev_z, prev_w, prev_g = zblk, wblk, gblk

    nc.compile()
    _NC_CACHE[reps] = nc
    return nc


def _stage(shard):
    """[32, 2000, 64] -> [128, 2000*16]; (b, j) -> partition b*4 + j//16."""
    return np.ascontiguousarray(
        shard.reshape(B, T, 4, 16).transpose(0, 2, 1, 3).reshape(P, T * CPT)
    )


def _unstage(arr):
    """[128, 2000*16] -> [32, 2000, 64] (inverse of _stage)."""
    return arr.reshape(B, 4, T, 16).transpose(0, 2, 1, 3).reshape(B, T, NSH)


def kernel(input_current):
    from concourse.bass_utils import run_bass_kernel_spmd

    input_current = np.asarray(input_current, dtype=F32)
    assert input_current.shape == (B, T, N)

    nc = _build_bass()
    in_maps = [
        {"inp": _stage(input_current[:, :, k * NSH:(k + 1) * NSH])}
        for k in range(NCORES)
    ]
    res = run_bass_kernel_spmd(nc, in_maps, list(range(NCORES)))

    spikes = np.empty((B, T, N), F32)
    volts = np.empty((B, T, N), F32)
    recov = np.empty((B, T, N), F32)
    for k in range(NCORES):
        sl = slice(k * NSH, (k + 1) * NSH)
        spikes[:, :, sl] = _unstage(res.results[k]["s_out"])
        volts[:, :, sl] = _unstage(res.results[k]["v_out"])
        recov[:, :, sl] = _unstage(res.results[k]["u_out"])
    return spikes, volts, recov

